# revision 1
# baseline (speedup 1.0000x reference)
"""GAT-style message passing kernel for Trainium2, data-parallel over batch.

Per batch b: e_k = leaky_relu((h*a_k) @ h^T), scores = select by adj value
(1..4 -> e_0..e_3, else -9e15), alpha = softmax(scores, -1), out = alpha @ h.

Key tricks:
  - e_k is symmetric, so alpha^T blocks come from PE-transposing exp(scores)
    blocks; no transpose of adj needed.
  - leaky_relu commutes with the select, applied once after combining.
  - softmax uses a constant shift (no row-max): scores sigma~16, max < 152
    needed for fp32 exp overflow => shift by 64 is safe.
  - matmuls in float32r (full PE rate at free dim >= 256).
  - masked select via copy_predicated with adj itself as the k=1 mask
    (nonzero == adj>=1) and is_ge masks for k=2..4; last-write-wins.
"""

from contextlib import ExitStack

import numpy as np

import concourse.bass as bass
from concourse import bacc
import concourse.mybir as mybir
import concourse.tile as tile
from concourse.bass_utils import run_bass_kernel_spmd
from concourse.masks import make_identity

B, N, D = 32, 512, 256
NCORES = 8
BPC = B // NCORES  # batches per core
P = 128
IB = N // P  # 4 i-blocks of 128 rows
DK = D // P  # 2 contraction subtiles
NEG = -9e15
SHIFT = 64.0
SLOPE = 0.2

f32 = mybir.dt.float32
f32r = mybir.dt.float32r
i32 = mybir.dt.int32
i8 = mybir.dt.int8

_CACHE = {}


def _build():
    nc = bacc.Bacc("TRN2", target_bir_lowering=False, debug=False)
    hid = nc.dram_tensor("hidden", [BPC, N, D], f32, kind="ExternalInput")
    hidT = nc.dram_tensor("hiddenT", [BPC, D, N], f32, kind="ExternalInput")
    adj = nc.dram_tensor("adj", [BPC, N, N], i32, kind="ExternalInput")
    a_cat = nc.dram_tensor("a_cat", [D, 4], f32, kind="ExternalInput")
    out = nc.dram_tensor("out", [BPC, N, D], f32, kind="ExternalOutput")

    with tile.TileContext(nc) as tc, ExitStack() as ctx:
        const = ctx.enter_context(tc.tile_pool(name="const", bufs=1))
        hpool = ctx.enter_context(tc.tile_pool(name="h", bufs=2))
        work = ctx.enter_context(tc.tile_pool(name="work", bufs=3))
        pse = ctx.enter_context(tc.tile_pool(name="pse", bufs=4, space="PSUM"))
        pst = ctx.enter_context(tc.tile_pool(name="pst", bufs=2, space="PSUM"))
        pso = ctx.enter_context(tc.tile_pool(name="pso", bufs=2, space="PSUM"))

        ident = const.tile([P, P], f32)
        make_identity(nc, ident)
        a_sb = const.tile([P, DK, 4], f32)
        nc.sync.dma_start(a_sb, a_cat.ap().rearrange("(dk p) k -> p dk k", p=P))
        neg_shift = const.tile([P, 1], f32)
        nc.vector.memset(neg_shift, -SHIFT)

        for b in range(BPC):
            # h in natural layout: [i_part, i_outer, d]
            h_sb = hpool.tile([P, IB, D], f32r, tag="h")
            nc.sync.dma_start(h_sb, hid.ap()[b].bitcast(f32r).rearrange("(io p) d -> p io d", p=P))

            # hT: [d_part, dk, i] loaded directly from host-transposed input
            hT = hpool.tile([P, DK, N], f32r, tag="hT")
            nc.sync.dma_start(
                hT, hidT.ap()[b].bitcast(f32r).rearrange("(dk p) i -> p dk i", p=P)
            )

            # hwT[k]: a_k-scaled hT  [d_part, dk*4+k, i]
            hwT = hpool.tile([P, DK * 4, N], f32r, tag="hwT")
            for dk in range(DK):
                for k in range(4):
                    nc.gpsimd.tensor_scalar_mul(
                        hwT[:, dk * 4 + k, :],
                        hT[:, dk, :],
                        a_sb[:, dk, k : k + 1],
                    )

            for c in range(IB):
                adj_sb = work.tile([P, N], i32, tag="adj")
                nc.sync.dma_start(adj_sb, adj.ap()[b, c * P : (c + 1) * P, :])

                # masks for k=2..4 (k=1 uses adj itself: nonzero == adj>=1)
                msk = work.tile([P, 3, N], i8, tag="msk")
                for t in range(3):
                    nc.gpsimd.tensor_scalar(
                        msk[:, t, :], adj_sb, t + 2, None, mybir.AluOpType.is_ge
                    )

                S = work.tile([P, N], f32, tag="S")
                nc.vector.memset(S, NEG)

                # raw scores e_k for this i-block: psum[i, j] over 4 banks
                e_ps = []
                for k in range(4):
                    e_k = pse.tile([P, N], f32, tag="e")
                    for dk in range(DK):
                        nc.tensor.matmul(
                            e_k,
                            lhsT=hwT[:, dk * 4 + k, c * P : (c + 1) * P],
                            rhs=hT[:, dk, :],
                            start=(dk == 0),
                            stop=(dk == DK - 1),
                        )
                    e_ps.append(e_k)

                # select: last-write-wins cascade of predicated copies
                nc.vector.copy_predicated(S, adj_sb, e_ps[0])
                for k in range(1, 4):
                    nc.vector.copy_predicated(S, msk[:, k - 1, :], e_ps[k])

                # leaky relu: S = max(S, 0.2*S)
                t02 = work.tile([P, N], f32, tag="t02")
                nc.gpsimd.tensor_scalar_mul(t02, S, SLOPE)
                nc.vector.tensor_tensor(S, S, t02, mybir.AluOpType.max)

                # p = exp(S - SHIFT), den = sum_j p  (fused accumulate)
                p_sb = work.tile([P, N], f32, tag="p")
                den = work.tile([P, 1], f32, tag="den")
                nc.scalar.activation(
                    p_sb,
                    S,
                    mybir.ActivationFunctionType.Exp,
                    bias=neg_shift,
                    scale=1.0,
                    accum_out=den,
                )
                r = work.tile([P, 1], f32, tag="r")
                nc.vector.reciprocal(r, den)

                # alphaT blocks via PE transpose (e_k symmetric trick)
                tp = pst.tile([P, N], f32, tag="tp")
                for jb in range(IB):
                    nc.tensor.transpose(
                        tp[:, jb * P : (jb + 1) * P],
                        p_sb[:, jb * P : (jb + 1) * P],
                        ident,
                    )
                alphaT = work.tile([P, N], f32r, tag="alphaT")
                nc.scalar.copy(alphaT, tp)

                # out block = (alphaT.T @ h) accumulated over j-subtiles
                o_ps = pso.tile([P, D], f32, tag="o")
                for jb in range(IB):
                    nc.tensor.matmul(
                        o_ps,
                        lhsT=alphaT[:, jb * P : (jb + 1) * P],
                        rhs=h_sb[:, jb, :],
                        start=(jb == 0),
                        stop=(jb == IB - 1),
                    )
                # normalize on copyback: out = psum * (1/den) per row
                o_sb = work.tile([P, D], f32, tag="o_sb")
                nc.scalar.activation(
                    o_sb,
                    o_ps,
                    mybir.ActivationFunctionType.Copy,
                    bias=0.0,
                    scale=r,
                )
                nc.sync.dma_start(out.ap()[b, c * P : (c + 1) * P, :], o_sb)

    nc.finalize()
    return nc


def kernel(hidden, adj, a_0, a_1, a_2, a_3, _trace=False):
    hidden = np.ascontiguousarray(hidden, dtype=np.float32)
    hiddenT = np.ascontiguousarray(hidden.transpose(0, 2, 1))
    adj = np.ascontiguousarray(adj, dtype=np.int32)
    a_cat = np.ascontiguousarray(
        np.concatenate([a_0, a_1, a_2, a_3], axis=1), dtype=np.float32
    )

    if "nc" not in _CACHE:
        _CACHE["nc"] = _build()
    nc = _CACHE["nc"]

    in_maps = []
    for core in range(NCORES):
        lo, hi = core * BPC, (core + 1) * BPC
        in_maps.append(
            {
                "hidden": hidden[lo:hi],
                "hiddenT": hiddenT[lo:hi],
                "adj": adj[lo:hi],
                "a_cat": a_cat,
            }
        )

    res = run_bass_kernel_spmd(
        nc, in_maps, core_ids=list(range(NCORES)), trace=_trace
    )
    out = np.concatenate([m["out"] for m in res.results], axis=0)
    if _trace:
        _CACHE["last_exec_time_ns"] = res.exec_time_ns
        _CACHE["last_results"] = res
    return out



# revision 4
# speedup vs baseline: 6.5843x; 6.5843x over previous
"""GAT-style message passing kernel for Trainium2, data-parallel over batch.

Per batch b: e_k = leaky_relu((h*a_k) @ h^T), scores = select by adj value
(1..4 -> e_0..e_3, else -9e15), alpha = softmax(scores, -1), out = alpha @ h.

This problem is wall-clock bound by the axon tunnel (host<->device transfer
at ~50 MB/s) and per-call jit re-tracing, not by device compute. So:
  - hidden ships as fp16 (8.4 MB instead of 16.8), upcast on device.
  - hiddenT is no longer shipped; hT is built on device via PE transposes.
  - adj ships nibble-packed int8 (4.2 MB instead of 33.5): byte j holds
    adj[i,j] in the low nibble and adj[i,j+256] in the high nibble.
  - output returns as fp16 (8.4 MB), upcast on host.
  - the jax/shard_map wrapper around the bass NEFF is built and jitted ONCE
    and cached; the donated output buffers are created on-device (jnp.zeros)
    instead of shipping 16.8 MB of host zeros per call.

Device-side algorithm (unchanged from the f32 version):
  - e_k is symmetric, so alpha^T blocks come from PE-transposing exp(scores)
    blocks; no transpose of adj needed.
  - leaky_relu commutes with the select, applied once after combining.
  - softmax uses a constant shift (no row-max): scores sigma~16, max < 152
    needed for fp32 exp overflow => shift by 64 is safe.
  - matmuls in float32r (full PE rate at free dim >= 256).
  - masked select via copy_predicated with adj itself as the k=1 mask
    (nonzero == adj>=1) and is_ge masks for k=2..4; last-write-wins.
"""

from contextlib import ExitStack

import numpy as np

import concourse.bass as bass
from concourse import bacc
import concourse.mybir as mybir
import concourse.tile as tile
from concourse.masks import make_identity

B, N, D = 32, 512, 256
NCORES = 8
BPC = B // NCORES  # batches per core
P = 128
IB = N // P  # 4 i-blocks of 128 rows
DK = D // P  # 2 contraction subtiles
NH = N // 2  # nibble-packed adj columns
NEG = -9e15
SHIFT = 64.0
SLOPE = 0.2

f16 = mybir.dt.float16
f32 = mybir.dt.float32
f32r = mybir.dt.float32r
i8 = mybir.dt.int8

_CACHE = {}


def _build():
    nc = bacc.Bacc("TRN2", target_bir_lowering=False, debug=False)
    hid = nc.dram_tensor("h16", [BPC, N, D], f16, kind="ExternalInput")
    adjp = nc.dram_tensor("adj_pk", [BPC, N, NH], i8, kind="ExternalInput")
    a_cat = nc.dram_tensor("a_cat", [D, 4], f32, kind="ExternalInput")
    out = nc.dram_tensor("out", [BPC, N, D], f16, kind="ExternalOutput")

    with tile.TileContext(nc) as tc, ExitStack() as ctx:
        const = ctx.enter_context(tc.tile_pool(name="const", bufs=1))
        hpool = ctx.enter_context(tc.tile_pool(name="h", bufs=2))
        work = ctx.enter_context(tc.tile_pool(name="work", bufs=3))
        pse = ctx.enter_context(tc.tile_pool(name="pse", bufs=4, space="PSUM"))
        pst = ctx.enter_context(tc.tile_pool(name="pst", bufs=2, space="PSUM"))
        pso = ctx.enter_context(tc.tile_pool(name="pso", bufs=2, space="PSUM"))

        ident = const.tile([P, P], f32)
        make_identity(nc, ident)
        a_sb = const.tile([P, DK, 4], f32)
        nc.sync.dma_start(a_sb, a_cat.ap().rearrange("(dk p) k -> p dk k", p=P))
        neg_shift = const.tile([P, 1], f32)
        nc.vector.memset(neg_shift, -SHIFT)

        for b in range(BPC):
            # h in fp16, upcast once to f32: [i_part, i_outer, d]
            h16_sb = hpool.tile([P, IB, D], f16, tag="h16")
            nc.sync.dma_start(h16_sb, hid.ap()[b].rearrange("(io p) d -> p io d", p=P))
            h_sb = hpool.tile([P, IB, D], f32r, tag="h")
            nc.scalar.copy(h_sb, h16_sb)

            # hT: [d_part, dk, i] via PE transposes of h_sb blocks
            hT = hpool.tile([P, DK, N], f32r, tag="hT")
            for dk in range(DK):
                tps = pst.tile([P, N], f32, tag="tp")
                for io in range(IB):
                    nc.tensor.transpose(
                        tps[:, io * P : (io + 1) * P],
                        h_sb[:, io, dk * P : (dk + 1) * P].bitcast(f32),
                        ident,
                    )
                nc.scalar.copy(hT[:, dk, :], tps)

            # hwT[k]: a_k-scaled hT  [d_part, dk*4+k, i]
            hwT = hpool.tile([P, DK * 4, N], f32r, tag="hwT")
            for dk in range(DK):
                for k in range(4):
                    nc.gpsimd.tensor_scalar_mul(
                        hwT[:, dk * 4 + k, :],
                        hT[:, dk, :],
                        a_sb[:, dk, k : k + 1],
                    )

            for c in range(IB):
                # adj block, nibble-packed: byte j = adj[i,j] | adj[i,j+256]<<4
                pk_sb = work.tile([P, NH], i8, tag="pk")
                nc.sync.dma_start(pk_sb, adjp.ap()[b, c * P : (c + 1) * P, :])
                adj_sb = work.tile([P, N], i8, tag="adj")
                nc.vector.tensor_scalar(
                    adj_sb[:, 0:NH], pk_sb, 0xF, None, mybir.AluOpType.bitwise_and
                )
                nc.vector.tensor_scalar(
                    adj_sb[:, NH:N], pk_sb, 4, None,
                    mybir.AluOpType.logical_shift_right,
                )

                # masks for k=2..4 (k=1 uses adj itself: nonzero == adj>=1)
                msk = work.tile([P, 3, N], i8, tag="msk")
                for t in range(3):
                    nc.gpsimd.tensor_scalar(
                        msk[:, t, :], adj_sb, t + 2, None, mybir.AluOpType.is_ge
                    )

                S = work.tile([P, N], f32, tag="S")
                nc.vector.memset(S, NEG)

                # raw scores e_k for this i-block: psum[i, j] over 4 banks
                e_ps = []
                for k in range(4):
                    e_k = pse.tile([P, N], f32, tag="e")
                    for dk in range(DK):
                        nc.tensor.matmul(
                            e_k,
                            lhsT=hwT[:, dk * 4 + k, c * P : (c + 1) * P],
                            rhs=hT[:, dk, :],
                            start=(dk == 0),
                            stop=(dk == DK - 1),
                        )
                    e_ps.append(e_k)

                # select: last-write-wins cascade of predicated copies
                nc.vector.copy_predicated(S, adj_sb, e_ps[0])
                for k in range(1, 4):
                    nc.vector.copy_predicated(S, msk[:, k - 1, :], e_ps[k])

                # leaky relu: S = max(S, 0.2*S)
                t02 = work.tile([P, N], f32, tag="t02")
                nc.gpsimd.tensor_scalar_mul(t02, S, SLOPE)
                nc.vector.tensor_tensor(S, S, t02, mybir.AluOpType.max)

                # p = exp(S - SHIFT), den = sum_j p  (fused accumulate)
                p_sb = work.tile([P, N], f32, tag="p")
                den = work.tile([P, 1], f32, tag="den")
                nc.scalar.activation(
                    p_sb,
                    S,
                    mybir.ActivationFunctionType.Exp,
                    bias=neg_shift,
                    scale=1.0,
                    accum_out=den,
                )
                r = work.tile([P, 1], f32, tag="r")
                nc.vector.reciprocal(r, den)

                # alphaT blocks via PE transpose (e_k symmetric trick)
                tp = pst.tile([P, N], f32, tag="tp")
                for jb in range(IB):
                    nc.tensor.transpose(
                        tp[:, jb * P : (jb + 1) * P],
                        p_sb[:, jb * P : (jb + 1) * P],
                        ident,
                    )
                alphaT = work.tile([P, N], f32r, tag="alphaT")
                nc.scalar.copy(alphaT, tp)

                # out block = (alphaT.T @ h) accumulated over j-subtiles
                o_ps = pso.tile([P, D], f32, tag="o")
                for jb in range(IB):
                    nc.tensor.matmul(
                        o_ps,
                        lhsT=alphaT[:, jb * P : (jb + 1) * P],
                        rhs=h_sb[:, jb, :],
                        start=(jb == 0),
                        stop=(jb == IB - 1),
                    )
                # normalize on copyback: out = psum * (1/den) per row, in fp16
                o_sb = work.tile([P, D], f16, tag="o_sb")
                nc.scalar.activation(
                    o_sb,
                    o_ps,
                    mybir.ActivationFunctionType.Copy,
                    bias=0.0,
                    scale=r,
                )
                nc.sync.dma_start(out.ap()[b, c * P : (c + 1) * P, :], o_sb)

    nc.finalize()
    return nc


def _get_state():
    if "st" in _CACHE:
        return _CACHE["st"]

    import jax
    import jax.numpy as jnp
    from jax.experimental.shard_map import shard_map
    from jax.sharding import Mesh, NamedSharding, PartitionSpec

    from concourse import bass2jax as b2j

    nc = _build()
    b2j.install_neuronx_cc_hook()

    # Collect input/output allocations in BIR order, like run_bass_via_pjrt.
    partition_name = nc.partition_id_tensor.name if nc.partition_id_tensor else None
    in_names: list[str] = []
    out_names: list[str] = []
    out_avals = []
    out_shapes: list[tuple] = []
    for alloc in nc.m.functions[0].allocations:
        if not isinstance(alloc, mybir.MemoryLocationSet):
            continue
        name = alloc.memorylocations[0].name
        if alloc.kind == "ExternalInput":
            if name != partition_name:
                in_names.append(name)
        elif alloc.kind == "ExternalOutput":
            shape = tuple(alloc.tensor_shape)
            dtype = mybir.dt.np(alloc.dtype)
            out_avals.append(jax.core.ShapedArray(shape, dtype))
            out_names.append(name)
            out_shapes.append((shape, dtype))
    n_params = len(in_names)
    n_outs = len(out_names)
    in_names.extend(out_names)
    if partition_name is not None:
        in_names.append(partition_name)

    def _body(*args):
        operands = list(args)
        if partition_name is not None:
            operands.append(b2j.partition_id_tensor())
        outs = b2j._bass_exec_p.bind(
            *operands,
            out_avals=tuple(out_avals),
            in_names=tuple(in_names),
            out_names=tuple(out_names),
            lowering_input_output_aliases=(),
            sim_require_finite=True,
            sim_require_nnan=True,
            nc=nc,
        )
        return tuple(outs)

    devices = jax.devices()[:NCORES]
    assert len(devices) == NCORES, f"need {NCORES} devices, got {len(jax.devices())}"
    mesh = Mesh(np.asarray(devices), ("core",))
    in_specs = (PartitionSpec("core"),) * (n_params + n_outs)
    out_specs = (PartitionSpec("core"),) * n_outs
    donate = tuple(range(n_params, n_params + n_outs))
    jitted = jax.jit(
        shard_map(
            _body, mesh=mesh, in_specs=in_specs, out_specs=out_specs, check_rep=False
        ),
        donate_argnums=donate,
        keep_unused=True,
    )

    zsh = NamedSharding(mesh, PartitionSpec("core"))

    def _zeros():
        return tuple(
            jnp.zeros((NCORES * s[0], *s[1:]), dt) for (s, dt) in out_shapes
        )

    make_zeros = jax.jit(_zeros, out_shardings=(zsh,) * n_outs)

    st = {"jitted": jitted, "make_zeros": make_zeros}
    _CACHE["st"] = st
    return st


def kernel(hidden, adj, a_0, a_1, a_2, a_3, _trace=False):
    h16 = np.ascontiguousarray(hidden, dtype=np.float16)
    a8 = np.asarray(adj).astype(np.int8)
    pk = np.ascontiguousarray(a8[:, :, :NH] | (a8[:, :, NH:] << 4))
    a_cat = np.ascontiguousarray(
        np.concatenate([a_0, a_1, a_2, a_3], axis=1), dtype=np.float32
    )
    a_tiled = np.tile(a_cat, (NCORES, 1))

    st = _get_state()
    zeros = st["make_zeros"]()
    out = st["jitted"](h16, pk, a_tiled, *zeros)[0]
    return np.asarray(out).astype(np.float32)
